# revision 31
# baseline (speedup 1.0000x reference)
"""ComplexAttention Trainium2 kernel (8 NeuronCores, Bass/Tile).

Problem: complex-valued QKV projections + causal attention, B=4, S=2048, D=1024.
  qr,qi / kr,ki / vr,vi = complex_linear(z, W*)          (z @ W^T per component)
  scores = (qr@kr^T + qi@ki^T) / sqrt(D), causal mask, softmax
  out = stack([attn@vr, attn@vi])                        -> [2, B, S, D]

Sharding (uniform SPMD, 8 cores): core c -> (batch b = c//2, d-half = c%2).
Each core computes the q/k projections for its batch restricted to its
512-wide dout half (weights arrive host-sliced), then a per-512-seq-chunk
packed AllGather within the batch pair assembles full q^T/k^T — the gathers
pipeline with the remaining projection work. The v projection covers only
the core's d-half. Every core computes full causal softmax statistics for
its batch and the attention output for its d-half (scores/exp duplicated
within the pair; the av matmuls and v are split).

All matmuls run as float32r (e8m10, round-to-nearest) at full PE rate with
fp32 PSUM accumulation; host pre-rounds the kernel inputs to fp32r.

Math note: softmax is computed without max-subtraction (scores are O(10), so
exp() is safe in fp32), as exp(s/sqrt(D)) normalized by a ones-matmul
denominator. Masking multiplies exp by a 0/1 mask on the 4 diagonal key
blocks of each 512-query macro block; off-diagonal blocks are either fully
kept or skipped entirely.
"""

import numpy as np

B, S, D = 4, 2048, 1024
P = 128
SQ = 512  # query macro-block width / matmul moving width
NDC = D // P  # 8 contraction chunks
NM = S // SQ  # 4 query macro blocks
N_CORES = 8
SCALE = float(D) ** -0.5

_COMPILED = {}


def _round_fp32r(x: np.ndarray) -> np.ndarray:
    """Round float32 -> float32r (e8m10) bit pattern, round-to-nearest-even."""
    x = np.ascontiguousarray(x, dtype=np.float32)
    try:
        from neuronxcc.starfish.support.dtype import static_cast_fp32_to_fp32r

        return np.asarray(static_cast_fp32_to_fp32r(x)).view(np.float32)
    except Exception:
        v = x.view(np.uint32)
        lsb = (v >> np.uint32(13)) & np.uint32(1)
        r = (v + np.uint32(0x0FFF) + lsb) & np.uint32(0xFFFFE000)
        return r.view(np.float32)


def _build_module(reps: int = 1):
    import concourse.tile as tile
    from concourse import bacc, mybir

    f32 = mybir.dt.float32
    f32r = mybir.dt.float32r
    EXP = mybir.ActivationFunctionType.Exp
    AX = mybir.AxisListType.X

    nc = bacc.Bacc("TRN2", target_bir_lowering=False, debug=False, num_devices=8)

    # ---- I/O ----
    # q/k/v projection weights arrive pre-sliced to this core's dout half.
    zr_d = nc.dram_tensor("zr", [D, S], f32r, kind="ExternalInput")
    zi_d = nc.dram_tensor("zi", [D, S], f32r, kind="ExternalInput")
    wq_r = nc.dram_tensor("wqr", [D, SQ], f32r, kind="ExternalInput")
    wq_i = nc.dram_tensor("wqi", [D, SQ], f32r, kind="ExternalInput")
    wk_r = nc.dram_tensor("wkr", [D, SQ], f32r, kind="ExternalInput")
    wk_i = nc.dram_tensor("wki", [D, SQ], f32r, kind="ExternalInput")
    wv_r = nc.dram_tensor("wvr", [D, SQ], f32r, kind="ExternalInput")
    wv_i = nc.dram_tensor("wvi", [D, SQ], f32r, kind="ExternalInput")
    dmask_d = nc.dram_tensor("dmask", [4, P, SQ], f32r, kind="ExternalInput")
    o_d = nc.dram_tensor("o", [2, S, SQ], f32, kind="ExternalOutput")

    # ---- DRAM scratch ----
    # q/k halves per 512-seq chunk, packed [qk, ci, 512, 512] so one
    # AllGather per chunk moves everything; gathered adds a leading pair
    # -half axis g: qkTs[sc][g, qk, ci, row, s] with dout = g*512 + row.
    qkTsh = nc.dram_tensor("qkTsh", [NM, 2, 2, SQ, SQ], f32r, kind="Internal")
    qkTs = nc.dram_tensor("qkTs", [NM, 2, 2, 2, SQ, SQ], f32r, kind="Internal")
    vs = [nc.dram_tensor(f"vs{c}", [S, SQ], f32r, kind="Internal") for c in range(2)]
    PAIRS = [[0, 1], [2, 3], [4, 5], [6, 7]]

    def load_z_chunk(zp, sc):
        """Load z^T tiles for one 512-seq chunk: (r, i, -i) x 8 dim-chunks."""
        ztr, zti, ztn = [], [], []
        for dc in range(NDC):
            tr = zp.tile([P, SQ], f32r, tag="zt", name=f"ztr{dc}")
            nc.sync.dma_start(
                tr[:], zr_d[dc * P : (dc + 1) * P, sc * SQ : (sc + 1) * SQ]
            )
            ti = zp.tile([P, SQ], f32r, tag="zt", name=f"zti{dc}")
            nc.sync.dma_start(
                ti[:], zi_d[dc * P : (dc + 1) * P, sc * SQ : (sc + 1) * SQ]
            )
            tn = zp.tile([P, SQ], f32r, tag="zt", name=f"ztn{dc}")
            nc.vector.tensor_scalar_mul(tn[:], ti[:], -1.0)
            ztr.append(tr)
            zti.append(ti)
            ztn.append(tn)
        return ztr, zti, ztn

    def load_w_chunks(wp, w_d, width, nm):
        """Load a weight matrix as 8 per-dc tiles [P, width] (one DMA each)."""
        tiles = []
        for dc in range(NDC):
            t = wp.tile([P, SQ], f32r, tag="w", name=f"{nm}{dc}")
            nc.sync.dma_start(t[:, 0:width], w_d[dc * P : (dc + 1) * P, 0:width])
            tiles.append(t)
        return tiles

    def emit_projections():
        with (
            tc.tile_pool(name="wpool", bufs=40) as wp,
            tc.tile_pool(name="zpool", bufs=32) as zp,
            tc.tile_pool(name="stg", bufs=4) as sp,
            tc.tile_pool(name="ppsum", bufs=4, space="PSUM") as pp,
        ):
            # ---- q and k projections (full S, this core's dout half) ----
            # q and k share each z chunk; one packed AllGather per chunk
            # pipelines the pair exchange with the remaining projection work.
            wq_t = (load_w_chunks(wp, wq_r, SQ, "wqr"), load_w_chunks(wp, wq_i, SQ, "wqi"))
            wk_t = (load_w_chunks(wp, wk_r, SQ, "wkr"), load_w_chunks(wp, wk_i, SQ, "wki"))
            for sc in range(NM):
                ztr, zti, ztn = load_z_chunk(zp, sc)
                for qk, (wr_t, wi_t) in enumerate((wq_t, wk_t)):
                    for db in range(SQ // P):
                        for ci, terms in enumerate(
                            (((wr_t, ztr), (wi_t, ztn)), ((wr_t, zti), (wi_t, ztr)))
                        ):
                            ps = pp.tile([P, SQ], f32, tag="pp", name="ps")
                            n = 0
                            for w_t, zt in terms:
                                for dc in range(NDC):
                                    nc.tensor.matmul(
                                        ps[:],
                                        w_t[dc][:, db * P : (db + 1) * P],
                                        zt[dc][:],
                                        start=(n == 0),
                                        stop=(n == 15),
                                    )
                                    n += 1
                            st = sp.tile([P, SQ], f32r, tag="st", name="st")
                            nc.vector.tensor_copy(st[:], ps[:])
                            nc.sync.dma_start(
                                qkTsh[sc, qk, ci, db * P : (db + 1) * P, :],
                                st[:],
                            )
                nc.gpsimd.collective_compute(
                    "AllGather",
                    mybir.AluOpType.bypass,
                    replica_groups=PAIRS,
                    ins=[qkTsh[sc].opt()],
                    outs=[qkTs[sc].opt()],
                )

            # ---- v projection (d-half, natural orientation) ----
            wvr_t = load_w_chunks(wp, wv_r, SQ, "wvr")
            wvi_t = load_w_chunks(wp, wv_i, SQ, "wvi")
            for sc in range(NM):
                ztr, zti, ztn = load_z_chunk(zp, sc)
                for sbl in range(SQ // P):
                    lo, hi = sbl * P, (sbl + 1) * P
                    for ci, terms in enumerate(
                        (((ztr, wvr_t), (ztn, wvi_t)), ((zti, wvr_t), (ztr, wvi_t)))
                    ):
                        ps = pp.tile([P, SQ], f32, tag="pp", name="ps")
                        n = 0
                        for zt, w_t in terms:
                            for dc in range(NDC):
                                nc.tensor.matmul(
                                    ps[:],
                                    zt[dc][:, lo:hi],
                                    w_t[dc][:, 0:SQ],
                                    start=(n == 0),
                                    stop=(n == 15),
                                )
                                n += 1
                        st = sp.tile([P, SQ], f32r, tag="st", name="st")
                        nc.vector.tensor_copy(st[:], ps[:])
                        nc.sync.dma_start(
                            vs[ci][(sc * 4 + sbl) * P : (sc * 4 + sbl + 1) * P, :],
                            st[:],
                        )

    def load_k_sc(pool, sc):
        """Load a whole 512-key chunk of k^T as 4 tiles [P, 4*SQ], one per
        (pair-half g, component ci); 2KB-contiguous DMA segments."""
        blk = {}
        for g in range(2):
            for ci in range(2):
                t = pool.tile([P, 4 * SQ], f32r, tag="kt", name=f"ksc{g}{ci}")
                nc.sync.dma_start(
                    t[:].rearrange("p (c t) -> p c t", c=4),
                    qkTs[sc, g, 1, ci].rearrange("(c p) t -> p c t", p=P),
                )
                blk[(g, ci)] = t
        return blk

    def emit_attention(mask_t, ones_t):
        with (
            tc.tile_pool(name="qtp", bufs=2) as qtp,
            tc.tile_pool(name="expp", bufs=20) as ep,
            tc.tile_pool(name="ktp", bufs=6) as ktp,
            tc.tile_pool(name="vtp", bufs=8) as vtp,
            tc.tile_pool(name="outp", bufs=4) as op,
            tc.tile_pool(name="smal", bufs=2) as smp,
            tc.tile_pool(name="spsum", bufs=2, space="PSUM") as sps,
            tc.tile_pool(name="dpsum", bufs=2, space="PSUM") as dps,
            tc.tile_pool(name="avpsum", bufs=4, space="PSUM") as avp,
        ):
            for m in range(NM):
                nkb = 4 * (m + 1)  # causal key blocks for this macro
                jm = 4 * m  # first diagonal key block
                qt = []
                for ci in range(2):
                    t = qtp.tile([P, NDC * SQ], f32r, tag=f"qt{ci}", name=f"qt{ci}")
                    for g in range(2):
                        nc.sync.dma_start(
                            t[:, g * 4 * SQ : (g + 1) * 4 * SQ].rearrange(
                                "p (c q) -> p c q", c=4
                            ),
                            qkTs[m, g, 0, ci].rearrange("(c p) q -> p c q", p=P),
                        )
                    qt.append(t)
                den_ps = dps.tile([P, 64], f32, tag="den", name="den_ps")
                expts = []
                for kb in range(nkb):
                    if kb % 4 == 0:
                        kblk = load_k_sc(ktp, kb // 4)
                    toff = (kb % 4) * P
                    ps = sps.tile([P, SQ], f32, tag="sc", name="ps")
                    n = 0
                    for ci in range(2):
                        for dc in range(NDC):
                            g, c4 = dc // 4, dc % 4
                            nc.tensor.matmul(
                                ps[:],
                                kblk[(g, ci)][
                                    :, c4 * SQ + toff : c4 * SQ + toff + P
                                ],
                                qt[ci][:, dc * SQ : (dc + 1) * SQ],
                                start=(n == 0),
                                stop=(n == 15),
                            )
                            n += 1
                    et = ep.tile([P, SQ], f32r, tag="et", name="et")
                    nc.scalar.activation(et[:], ps[:], EXP, scale=SCALE)
                    if kb >= jm:
                        nc.vector.tensor_mul(et[:], et[:], mask_t[kb - jm][:])
                    expts.append(et)
                    for sub in range(max(0, kb - jm), 4):
                        c = sub * 16 + kb
                        nc.tensor.matmul(
                            den_ps[:, c : c + 1],
                            et[:, sub * P : (sub + 1) * P].bitcast(f32),
                            ones_t[:],
                            start=True,
                            stop=True,
                        )
                den_sb = smp.tile([P, 4], f32, tag="densb", name="den_sb")
                for sub in range(4):
                    nc.vector.reduce_sum(
                        den_sb[:, sub : sub + 1],
                        den_ps[:, sub * 16 : sub * 16 + jm + sub + 1],
                        axis=AX,
                    )
                recip = smp.tile([P, 4], f32, tag="recip", name="recip")
                nc.vector.reciprocal(recip[:], den_sb[:])

                for pair in range(2):
                    subs = (2 * pair, 2 * pair + 1)
                    j_hi = jm + subs[1]
                    av = {}
                    for sl in range(2):
                        for ci in range(2):
                            av[(sl, ci)] = avp.tile(
                                [P, SQ], f32, tag="av", name=f"av{sl}{ci}"
                            )
                    for kb in range(j_hi + 1):
                        vt = []
                        for ci in range(2):
                            t = vtp.tile([P, SQ], f32r, tag="vt", name=f"vt{ci}")
                            nc.sync.dma_start(t[:], vs[ci][kb * P : (kb + 1) * P, :])
                            vt.append(t)
                        for sl, sub in enumerate(subs):
                            j = jm + sub
                            if kb > j:
                                continue
                            for ci in range(2):
                                nc.tensor.matmul(
                                    av[(sl, ci)][:],
                                    expts[kb][:, sub * P : (sub + 1) * P],
                                    vt[ci][:],
                                    start=(kb == 0),
                                    stop=(kb == j),
                                )
                    for sl, sub in enumerate(subs):
                        row = m * SQ + sub * P
                        for ci in range(2):
                            ot = op.tile([P, SQ], f32, tag="ot", name="ot")
                            nc.vector.tensor_scalar_mul(
                                ot[:], av[(sl, ci)][:], recip[:, sub : sub + 1]
                            )
                            nc.sync.dma_start(o_d[ci, row : row + P, :], ot[:])

    with tile.TileContext(nc) as tc:
        with tc.tile_pool(name="const", bufs=1) as cp:
            mask_t = []
            for idx in range(4):
                mt = cp.tile([P, SQ], f32r, tag=f"mask{idx}", name=f"mask{idx}")
                nc.sync.dma_start(mt[:], dmask_d[idx])
                mask_t.append(mt)
            ones_t = cp.tile([P, 1], f32, tag="ones", name="ones_t")
            nc.vector.memset(ones_t[:], 1.0)
            for _rep in range(reps):
                emit_projections()
                emit_attention(mask_t, ones_t)

    nc.compile()
    return nc


def get_module(reps: int = 1):
    key = ("nc", reps)
    if key not in _COMPILED:
        _COMPILED[key] = _build_module(reps)
    return _COMPILED[key]


def prepare_in_maps(z_real, z_imag, wq_r, wq_i, wk_r, wk_i, wv_r, wv_i, mask):
    """Host-side sharding/layout prep -> list of per-core input dicts."""
    r = _round_fp32r
    zT_r = [r(np.asarray(z_real)[b].T) for b in range(B)]
    zT_i = [r(np.asarray(z_imag)[b].T) for b in range(B)]
    # weights: torch Linear W is [out, in]; matmuls want W^T = [in, out]
    wqr_T = r(np.asarray(wq_r).T)
    wqi_T = r(np.asarray(wq_i).T)
    wkr_T = r(np.asarray(wk_r).T)
    wki_T = r(np.asarray(wk_i).T)
    wvr_T = r(np.asarray(wv_r).T)
    wvi_T = r(np.asarray(wv_i).T)
    # diagonal-block masks from the provided mask (macro 3 as representative)
    mask = np.asarray(mask)
    dmask = np.zeros((4, P, SQ), np.float32)
    g0 = 3 * SQ
    for idx in range(4):
        k0 = (12 + idx) * P
        dmask[idx] = (mask[g0 : g0 + SQ, k0 : k0 + P] != 0).T.astype(np.float32)
    in_maps = []
    for c in range(N_CORES):
        b, dh = c // 2, c % 2
        half = slice(dh * SQ, (dh + 1) * SQ)
        in_maps.append(
            {
                "zr": zT_r[b],
                "zi": zT_i[b],
                "wqr": np.ascontiguousarray(wqr_T[:, half]),
                "wqi": np.ascontiguousarray(wqi_T[:, half]),
                "wkr": np.ascontiguousarray(wkr_T[:, half]),
                "wki": np.ascontiguousarray(wki_T[:, half]),
                "wvr": np.ascontiguousarray(wvr_T[:, half]),
                "wvi": np.ascontiguousarray(wvi_T[:, half]),
                "dmask": dmask,
            }
        )
    return in_maps


def assemble_output(results):
    """Per-core outputs [2, S, 512] -> full [2, B, S, D]."""
    out = np.empty((2, B, S, D), np.float32)
    for c in range(N_CORES):
        b, dh = c // 2, c % 2
        out[:, b, :, dh * SQ : (dh + 1) * SQ] = results[c]["o"]
    return out


def kernel(**inputs) -> np.ndarray:
    from concourse.bass_utils import run_bass_kernel_spmd

    nc = get_module()
    in_maps = prepare_in_maps(**inputs)
    res = run_bass_kernel_spmd(nc, in_maps, core_ids=list(range(N_CORES)))
    return assemble_output(res.results)


# revision 34
# speedup vs baseline: 1.1972x; 1.1972x over previous
"""ComplexAttention Trainium2 kernel (8 NeuronCores, Bass/Tile).

Problem: complex-valued QKV projections + causal attention, B=4, S=2048, D=1024.
  qr,qi / kr,ki / vr,vi = complex_linear(z, W*)          (z @ W^T per component)
  scores = (qr@kr^T + qi@ki^T) / sqrt(D), causal mask, softmax
  out = stack([attn@vr, attn@vi])                        -> [2, B, S, D]

Sharding (uniform SPMD, 8 cores): core c -> (batch b = c//2, d-half = c%2).
Each core computes the q/k projections for its batch restricted to its
512-wide dout half (weights arrive host-sliced), then a per-512-seq-chunk
packed AllGather within the batch pair assembles full q^T/k^T — the gathers
pipeline with the remaining projection work. The v projection covers only
the core's d-half. Every core computes full causal softmax statistics for
its batch and the attention output for its d-half (scores/exp duplicated
within the pair; the av matmuls and v are split).

All matmuls run as float32r (e8m10, round-to-nearest) at full PE rate with
fp32 PSUM accumulation; host pre-rounds the kernel inputs to fp32r.

Math note: softmax is computed without max-subtraction (scores are O(10), so
exp() is safe in fp32), as exp(s/sqrt(D)) normalized by a ones-matmul
denominator. Masking multiplies exp by a 0/1 mask on the 4 diagonal key
blocks of each 512-query macro block; off-diagonal blocks are either fully
kept or skipped entirely.
"""

import numpy as np

B, S, D = 4, 2048, 1024
P = 128
SQ = 512  # query macro-block width / matmul moving width
NDC = D // P  # 8 contraction chunks
NM = S // SQ  # 4 query macro blocks
N_CORES = 8
SCALE = float(D) ** -0.5

_COMPILED = {}


def _round_fp32r(x: np.ndarray) -> np.ndarray:
    """Round float32 -> float32r (e8m10) bit pattern, round-to-nearest-even."""
    x = np.ascontiguousarray(x, dtype=np.float32)
    try:
        from neuronxcc.starfish.support.dtype import static_cast_fp32_to_fp32r

        return np.asarray(static_cast_fp32_to_fp32r(x)).view(np.float32)
    except Exception:
        v = x.view(np.uint32)
        lsb = (v >> np.uint32(13)) & np.uint32(1)
        r = (v + np.uint32(0x0FFF) + lsb) & np.uint32(0xFFFFE000)
        return r.view(np.float32)


def _build_module(reps: int = 1):
    import concourse.tile as tile
    from concourse import bacc, mybir

    f32 = mybir.dt.float32
    f32r = mybir.dt.float32r
    EXP = mybir.ActivationFunctionType.Exp
    AX = mybir.AxisListType.X

    nc = bacc.Bacc("TRN2", target_bir_lowering=False, debug=False, num_devices=8)

    # ---- I/O ----
    # q/k/v projection weights arrive pre-sliced to this core's dout half.
    zr_d = nc.dram_tensor("zr", [D, S], f32r, kind="ExternalInput")
    zi_d = nc.dram_tensor("zi", [D, S], f32r, kind="ExternalInput")
    wq_r = nc.dram_tensor("wqr", [D, SQ], f32r, kind="ExternalInput")
    wq_i = nc.dram_tensor("wqi", [D, SQ], f32r, kind="ExternalInput")
    wk_r = nc.dram_tensor("wkr", [D, SQ], f32r, kind="ExternalInput")
    wk_i = nc.dram_tensor("wki", [D, SQ], f32r, kind="ExternalInput")
    wv_r = nc.dram_tensor("wvr", [D, SQ], f32r, kind="ExternalInput")
    wv_i = nc.dram_tensor("wvi", [D, SQ], f32r, kind="ExternalInput")
    dmask_d = nc.dram_tensor("dmask", [4, P, SQ], f32r, kind="ExternalInput")
    o_d = nc.dram_tensor("o", [2, S, SQ], f32, kind="ExternalOutput")

    # ---- DRAM scratch ----
    # q/k halves per 512-seq chunk, packed [qk, ci, 512, 512] so one
    # AllGather per chunk moves everything; gathered adds a leading pair
    # -half axis g: qkTs[sc][g, qk, ci, row, s] with dout = g*512 + row.
    qkTsh = nc.dram_tensor("qkTsh", [NM, 2, 2, SQ, SQ], f32r, kind="Internal")
    qkTs = nc.dram_tensor("qkTs", [NM, 2, 2, 2, SQ, SQ], f32r, kind="Internal")
    vs = [nc.dram_tensor(f"vs{c}", [S, SQ], f32r, kind="Internal") for c in range(2)]
    PAIRS = [[0, 1], [2, 3], [4, 5], [6, 7]]

    def load_z_chunk(zp, sc):
        """Load z^T tiles for one 512-seq chunk: (r, i, -i) x 8 dim-chunks."""
        ztr, zti, ztn = [], [], []
        for dc in range(NDC):
            tr = zp.tile([P, SQ], f32r, tag="zt", name=f"ztr{dc}")
            nc.sync.dma_start(
                tr[:], zr_d[dc * P : (dc + 1) * P, sc * SQ : (sc + 1) * SQ]
            )
            ti = zp.tile([P, SQ], f32r, tag="zt", name=f"zti{dc}")
            nc.sync.dma_start(
                ti[:], zi_d[dc * P : (dc + 1) * P, sc * SQ : (sc + 1) * SQ]
            )
            tn = zp.tile([P, SQ], f32r, tag="zt", name=f"ztn{dc}")
            nc.vector.tensor_scalar_mul(tn[:], ti[:], -1.0)
            ztr.append(tr)
            zti.append(ti)
            ztn.append(tn)
        return ztr, zti, ztn

    def load_w_chunks(wp, w_d, width, nm):
        """Load a weight matrix as 8 per-dc tiles [P, width] (one DMA each)."""
        tiles = []
        for dc in range(NDC):
            t = wp.tile([P, SQ], f32r, tag="w", name=f"{nm}{dc}")
            nc.sync.dma_start(t[:, 0:width], w_d[dc * P : (dc + 1) * P, 0:width])
            tiles.append(t)
        return tiles

    def emit_projections():
        with (
            tc.tile_pool(name="wpool", bufs=50) as wp,
            tc.tile_pool(name="zpool", bufs=28) as zp,
            tc.tile_pool(name="stg", bufs=4) as sp,
            tc.tile_pool(name="ppsum", bufs=4, space="PSUM") as pp,
        ):
            # ---- q and k projections (full S, this core's dout half) ----
            # q and k share each z chunk; one packed AllGather per chunk
            # pipelines the pair exchange with the remaining projection work.
            wq_t = (load_w_chunks(wp, wq_r, SQ, "wqr"), load_w_chunks(wp, wq_i, SQ, "wqi"))
            wk_t = (load_w_chunks(wp, wk_r, SQ, "wkr"), load_w_chunks(wp, wk_i, SQ, "wki"))
            wvr_t = load_w_chunks(wp, wv_r, SQ, "wvr")
            wvi_t = load_w_chunks(wp, wv_i, SQ, "wvi")
            for sc in range(NM):
                ztr, zti, ztn = load_z_chunk(zp, sc)
                for qk, (wr_t, wi_t) in enumerate((wq_t, wk_t)):
                    for db in range(SQ // P):
                        for ci, terms in enumerate(
                            (((wr_t, ztr), (wi_t, ztn)), ((wr_t, zti), (wi_t, ztr)))
                        ):
                            ps = pp.tile([P, SQ], f32, tag="pp", name="ps")
                            n = 0
                            for w_t, zt in terms:
                                for dc in range(NDC):
                                    nc.tensor.matmul(
                                        ps[:],
                                        w_t[dc][:, db * P : (db + 1) * P],
                                        zt[dc][:],
                                        start=(n == 0),
                                        stop=(n == 15),
                                    )
                                    n += 1
                            st = sp.tile([P, SQ], f32r, tag="st", name="st")
                            nc.vector.tensor_copy(st[:], ps[:])
                            nc.sync.dma_start(
                                qkTsh[sc, qk, ci, db * P : (db + 1) * P, :],
                                st[:],
                            )
                nc.gpsimd.collective_compute(
                    "AllGather",
                    mybir.AluOpType.bypass,
                    replica_groups=PAIRS,
                    ins=[qkTsh[sc].opt()],
                    outs=[qkTs[sc].opt()],
                )

                # ---- v projection for this chunk (shares the z tiles) ----
                for sbl in range(SQ // P):
                    lo, hi = sbl * P, (sbl + 1) * P
                    for ci, terms in enumerate(
                        (((ztr, wvr_t), (ztn, wvi_t)), ((zti, wvr_t), (ztr, wvi_t)))
                    ):
                        ps = pp.tile([P, SQ], f32, tag="pp", name="ps")
                        n = 0
                        for zt, w_t in terms:
                            for dc in range(NDC):
                                nc.tensor.matmul(
                                    ps[:],
                                    zt[dc][:, lo:hi],
                                    w_t[dc][:, 0:SQ],
                                    start=(n == 0),
                                    stop=(n == 15),
                                )
                                n += 1
                        st = sp.tile([P, SQ], f32r, tag="st", name="st")
                        nc.vector.tensor_copy(st[:], ps[:])
                        nc.sync.dma_start(
                            vs[ci][(sc * 4 + sbl) * P : (sc * 4 + sbl + 1) * P, :],
                            st[:],
                        )

    def load_kt_tile(pool, ci, kb, bufs_name):
        """Load one key block [P, NDC*P] = [d_local, (dc, t)]; dc = g*4 + c."""
        t = pool.tile([P, NDC * P], f32r, tag="kt", name=bufs_name)
        sc, toff = kb // 4, (kb % 4) * P
        for g in range(2):
            nc.sync.dma_start(
                t[:, g * 4 * P : (g + 1) * 4 * P].rearrange("p (c t) -> p c t", c=4),
                qkTs[sc, g, 1, ci, :, toff : toff + P].rearrange(
                    "(c p) t -> p c t", p=P
                ),
            )
        return t

    def emit_attention(mask_t, ones_t):
        with (
            tc.tile_pool(name="qtp", bufs=2) as qtp,
            tc.tile_pool(name="expp", bufs=20) as ep,
            tc.tile_pool(name="ktcp", bufs=8) as ktcp,
            tc.tile_pool(name="ktp", bufs=4) as ktp,
            tc.tile_pool(name="vtp", bufs=10) as vtp,
            tc.tile_pool(name="outp", bufs=4) as op,
            tc.tile_pool(name="smal", bufs=2) as smp,
            tc.tile_pool(name="spsum", bufs=2, space="PSUM") as sps,
            tc.tile_pool(name="dpsum", bufs=1, space="PSUM") as dps,
            tc.tile_pool(name="avpsum", bufs=5, space="PSUM") as avp,
        ):
            # key blocks 0-3 are used by every macro: cache them in SBUF
            ktc = {
                (kb, ci): load_kt_tile(ktcp, ci, kb, f"ktc{kb}{ci}")
                for kb in range(4)
                for ci in range(2)
            }
            for m in range(NM):
                nkb = 4 * (m + 1)  # causal key blocks for this macro
                jm = 4 * m  # first diagonal key block
                qt = []
                for ci in range(2):
                    t = qtp.tile([P, NDC * SQ], f32r, tag=f"qt{ci}", name=f"qt{ci}")
                    for g in range(2):
                        nc.sync.dma_start(
                            t[:, g * 4 * SQ : (g + 1) * 4 * SQ].rearrange(
                                "p (c q) -> p c q", c=4
                            ),
                            qkTs[m, g, 0, ci].rearrange("(c p) q -> p c q", p=P),
                        )
                    qt.append(t)
                den_ps = dps.tile([P, 64], f32, tag="den", name="den_ps")
                expts = []
                for kb in range(nkb):
                    if kb < 4:
                        kt = [ktc[(kb, 0)], ktc[(kb, 1)]]
                    else:
                        kt = [
                            load_kt_tile(ktp, ci, kb, f"kt{ci}") for ci in range(2)
                        ]
                    ps = sps.tile([P, SQ], f32, tag="sc", name="ps")
                    n = 0
                    for ci in range(2):
                        for dc in range(NDC):
                            nc.tensor.matmul(
                                ps[:],
                                kt[ci][:, dc * P : (dc + 1) * P],
                                qt[ci][:, dc * SQ : (dc + 1) * SQ],
                                start=(n == 0),
                                stop=(n == 15),
                            )
                            n += 1
                    et = ep.tile([P, SQ], f32r, tag="et", name="et")
                    nc.scalar.activation(et[:], ps[:], EXP, scale=SCALE)
                    if kb >= jm:
                        nc.vector.tensor_mul(et[:], et[:], mask_t[kb - jm][:])
                    expts.append(et)
                    for sub in range(max(0, kb - jm), 4):
                        c = sub * 16 + kb
                        nc.tensor.matmul(
                            den_ps[:, c : c + 1],
                            et[:, sub * P : (sub + 1) * P].bitcast(f32),
                            ones_t[:],
                            start=True,
                            stop=True,
                        )
                den_sb = smp.tile([P, 4], f32, tag="densb", name="den_sb")
                for sub in range(4):
                    nc.vector.reduce_sum(
                        den_sb[:, sub : sub + 1],
                        den_ps[:, sub * 16 : sub * 16 + jm + sub + 1],
                        axis=AX,
                    )
                recip = smp.tile([P, 4], f32, tag="recip", name="recip")
                nc.vector.reciprocal(recip[:], den_sb[:])

                for pair in range(2):
                    subs = (2 * pair, 2 * pair + 1)
                    j_hi = jm + subs[1]
                    av = {}
                    for sl in range(2):
                        for ci in range(2):
                            av[(sl, ci)] = avp.tile(
                                [P, SQ], f32, tag="av", name=f"av{sl}{ci}"
                            )
                    for kb in range(j_hi + 1):
                        vt = []
                        for ci in range(2):
                            t = vtp.tile([P, SQ], f32r, tag="vt", name=f"vt{ci}")
                            nc.sync.dma_start(t[:], vs[ci][kb * P : (kb + 1) * P, :])
                            vt.append(t)
                        for sl, sub in enumerate(subs):
                            j = jm + sub
                            if kb > j:
                                continue
                            for ci in range(2):
                                nc.tensor.matmul(
                                    av[(sl, ci)][:],
                                    expts[kb][:, sub * P : (sub + 1) * P],
                                    vt[ci][:],
                                    start=(kb == 0),
                                    stop=(kb == j),
                                )
                    for sl, sub in enumerate(subs):
                        row = m * SQ + sub * P
                        for ci in range(2):
                            ot = op.tile([P, SQ], f32, tag="ot", name="ot")
                            nc.vector.tensor_scalar_mul(
                                ot[:], av[(sl, ci)][:], recip[:, sub : sub + 1]
                            )
                            nc.sync.dma_start(o_d[ci, row : row + P, :], ot[:])

    with tile.TileContext(nc) as tc:
        with tc.tile_pool(name="const", bufs=1) as cp:
            mask_t = []
            for idx in range(4):
                mt = cp.tile([P, SQ], f32r, tag=f"mask{idx}", name=f"mask{idx}")
                nc.sync.dma_start(mt[:], dmask_d[idx])
                mask_t.append(mt)
            ones_t = cp.tile([P, 1], f32, tag="ones", name="ones_t")
            nc.vector.memset(ones_t[:], 1.0)
            for _rep in range(reps):
                emit_projections()
                emit_attention(mask_t, ones_t)

    nc.compile()
    return nc


def get_module(reps: int = 1):
    key = ("nc", reps)
    if key not in _COMPILED:
        _COMPILED[key] = _build_module(reps)
    return _COMPILED[key]


def prepare_in_maps(z_real, z_imag, wq_r, wq_i, wk_r, wk_i, wv_r, wv_i, mask):
    """Host-side sharding/layout prep -> list of per-core input dicts."""
    r = _round_fp32r
    zT_r = [r(np.asarray(z_real)[b].T) for b in range(B)]
    zT_i = [r(np.asarray(z_imag)[b].T) for b in range(B)]
    # weights: torch Linear W is [out, in]; matmuls want W^T = [in, out]
    wqr_T = r(np.asarray(wq_r).T)
    wqi_T = r(np.asarray(wq_i).T)
    wkr_T = r(np.asarray(wk_r).T)
    wki_T = r(np.asarray(wk_i).T)
    wvr_T = r(np.asarray(wv_r).T)
    wvi_T = r(np.asarray(wv_i).T)
    # diagonal-block masks from the provided mask (macro 3 as representative)
    mask = np.asarray(mask)
    dmask = np.zeros((4, P, SQ), np.float32)
    g0 = 3 * SQ
    for idx in range(4):
        k0 = (12 + idx) * P
        dmask[idx] = (mask[g0 : g0 + SQ, k0 : k0 + P] != 0).T.astype(np.float32)
    in_maps = []
    for c in range(N_CORES):
        b, dh = c // 2, c % 2
        half = slice(dh * SQ, (dh + 1) * SQ)
        in_maps.append(
            {
                "zr": zT_r[b],
                "zi": zT_i[b],
                "wqr": np.ascontiguousarray(wqr_T[:, half]),
                "wqi": np.ascontiguousarray(wqi_T[:, half]),
                "wkr": np.ascontiguousarray(wkr_T[:, half]),
                "wki": np.ascontiguousarray(wki_T[:, half]),
                "wvr": np.ascontiguousarray(wvr_T[:, half]),
                "wvi": np.ascontiguousarray(wvi_T[:, half]),
                "dmask": dmask,
            }
        )
    return in_maps


def assemble_output(results):
    """Per-core outputs [2, S, 512] -> full [2, B, S, D]."""
    out = np.empty((2, B, S, D), np.float32)
    for c in range(N_CORES):
        b, dh = c // 2, c % 2
        out[:, b, :, dh * SQ : (dh + 1) * SQ] = results[c]["o"]
    return out


def kernel(**inputs) -> np.ndarray:
    from concourse.bass_utils import run_bass_kernel_spmd

    nc = get_module()
    in_maps = prepare_in_maps(**inputs)
    res = run_bass_kernel_spmd(nc, in_maps, core_ids=list(range(N_CORES)))
    return assemble_output(res.results)
